# revision 1
# baseline (speedup 1.0000x reference)
"""GQA attention (RoPE, causal) + o_proj on 8 Trainium2 NeuronCores.

Sharding: 8 cores = 2 batches (DP) x 4 kv-head-pairs (TP over GQA groups).
Per core: hsT[batch] [D,S] (host-pretransposed), Wq slice [D,512] (8 q heads),
Wk/Wv slice [D,128] (2 kv heads), Wo slice [512,D]. Core computes its heads'
attention and a partial o_proj output [S,D]; host sums 4 partials per batch.

Kernel dataflow (per core; fp16 matmul operands, fp32 PSUM accumulate):
  1. q/k/v = hsT.T @ W (natural layout)
  2. RoPE in natural layout (host passes replicated/sign-folded trig tables)
  3. PE-transpose q,k -> qT [hd,s]; k2T has the kv head replicated on both
     64-partition halves for 2-head row-packed score matmuls
  4. scores^T[k,q] = k2T.T @ qT per 128k x 512q block, causal blocks only,
     two heads packed via tile_position row groups, two k-tiles per PSUM
     [128,1024] tile so exp amortizes ACT instruction overhead
  5. P^T = exp(scores^T) -> fp16 (ACT); 0/1 causal mask multiplied on diag
     blocks (DVE, fp16 4x)
  6. A^T_aug = [V|1].T @ P^T (fp16) accumulated over k tiles in PSUM; row 64
     is the softmax denominator; normalize via fast-reciprocal +
     gpsimd partition_broadcast + DVE multiply
  7. y = A @ Wo via lhsT=A^T (fp16), accumulate 4 feature tiles, DMA out
"""
import sys
import numpy as np

sys.path.insert(0, "/opt/trn_rl_repo")

B, S, D = 2, 2048, 2048
H, KVH, HD = 32, 8, 64
SCALE = HD ** -0.5
P = 128


def build_nc(S=S, D=D, LQ=8, LKV=2, HD=64):
    import concourse.bacc as bacc
    import concourse.mybir as mybir
    from concourse import tile
    from concourse.masks import make_identity

    f32 = mybir.dt.float32
    f16 = mybir.dt.float16

    QF = LQ * HD          # local q features (512)
    KF = LKV * HD         # local kv features (128)
    FT = QF // P          # q feature tiles = head pairs (4)
    DT = D // P           # contraction tiles (16)
    ST = S // P           # sequence tiles (16)
    NJ = S // 512         # q supertiles (4)
    GRP = LQ // LKV       # q heads per kv head (4)
    VW = HD + 1           # v + ones column (65)
    Exp = mybir.ActivationFunctionType.Exp

    nc = bacc.Bacc(None, target_bir_lowering=False)
    hsT = nc.declare_dram_parameter("hsT", [D, S], f16, isOutput=False)
    wq = nc.declare_dram_parameter("wq", [D, QF], f16, isOutput=False)
    wkv = nc.declare_dram_parameter("wkv", [D, 2 * KF], f16, isOutput=False)
    wo = nc.declare_dram_parameter("wo", [QF, D], f16, isOutput=False)
    cosq = nc.declare_dram_parameter("cosq", [S, QF], f32, isOutput=False)
    sinq = nc.declare_dram_parameter("sinq", [S, QF], f32, isOutput=False)
    cosk = nc.declare_dram_parameter("cosk", [S, KF], f32, isOutput=False)
    sink = nc.declare_dram_parameter("sink", [S, KF], f32, isOutput=False)
    out = nc.declare_dram_parameter("out", [S, D], f32, isOutput=True)

    hsT_r = hsT.rearrange("(dt p) s -> p dt s", p=P)
    wq_t = wq.rearrange("(dt p) f -> p dt f", p=P)
    wkv_t = wkv.rearrange("(dt p) f -> p dt f", p=P)
    wo_t = wo.rearrange("(ft p) d -> p ft d", p=P)
    out_t = out.rearrange("(st p) d -> p st d", p=P)

    with tile.TileContext(nc) as tc:
        with tc.tile_pool(name="persist", bufs=1) as persist:
            ident = persist.tile([P, P], f16)
            maskb = persist.tile([P, 4, 512], f16)
            qT = persist.tile([P, FT, S], f16)
            k2T = persist.tile([P, LKV, S], f16)
            vaug = persist.tile([P, ST, LKV * VW], f16)
            with tc.tile_pool(name="init", bufs=1) as init_p:
                ident_f = init_p.tile([P, P], f32)
                make_identity(nc, ident_f[:])
                nc.vector.tensor_copy(ident[:], ident_f[:])
                # maskb[kr, m, qc] = 1 where visible (qc >= 128*m+kr) else 0
                mask_f = init_p.tile([P, 4, 512], f32)
                for m in range(4):
                    nc.gpsimd.memset(mask_f[:, m, :], 1.0)
                    nc.gpsimd.affine_select(
                        out=mask_f[:, m, :], in_=mask_f[:, m, :],
                        compare_op=mybir.AluOpType.is_ge, fill=0.0,
                        base=-P * m, pattern=[[1, 512]], channel_multiplier=-1,
                    )
                nc.vector.tensor_copy(maskb[:], mask_f[:])
                ones_f = init_p.tile([P, 1], f32)
                nc.gpsimd.memset(ones_f[:], 1.0)
                for lkv in range(LKV):
                    for st_i in range(ST):
                        nc.vector.tensor_copy(
                            vaug[:, st_i, lkv * VW + HD:lkv * VW + HD + 1],
                            ones_f[:])

            # ---------------- phase 1: projections + RoPE + transposes ----
            with (
                tc.tile_pool(name="wq_p", bufs=1) as wq_p,
                tc.tile_pool(name="wkv_p", bufs=1) as wkv_p,
                tc.tile_pool(name="hsT", bufs=2) as hsT_p,
                tc.tile_pool(name="trig", bufs=2) as trig_p,
                tc.tile_pool(name="rope", bufs=2) as rope_p,
                tc.tile_pool(name="ps_tr", bufs=4, space="PSUM") as ps_tr,
                tc.tile_pool(name="ps_q", bufs=2, space="PSUM") as ps_q,
                tc.tile_pool(name="ps_kv", bufs=2, space="PSUM") as ps_kv,
            ):
                wq_sb = wq_p.tile([P, DT, QF], f16)
                nc.sync.dma_start(out=wq_sb[:], in_=wq_t)
                wkv_sb = wkv_p.tile([P, DT, 2 * KF], f16)
                nc.sync.dma_start(out=wkv_sb[:], in_=wkv_t)

                for blk in range(S // 256):
                    hsT_b = hsT_p.tile([P, DT, 256], f16, tag="hsT")
                    nc.sync.dma_start(
                        out=hsT_b[:],
                        in_=hsT_r[:, :, blk * 256:(blk + 1) * 256])
                    for j in range(2):
                        si = blk * 2 + j
                        # ---- q = hs @ Wq (natural), RoPE, transpose ----
                        psq = ps_q.tile([P, QF], f32, tag="psq")
                        for dt in range(DT):
                            nc.tensor.matmul(
                                psq[:], hsT_b[:, dt, j * P:(j + 1) * P],
                                wq_sb[:, dt, :],
                                start=(dt == 0), stop=(dt == DT - 1))
                        cq = trig_p.tile([P, QF], f32, tag="cq")
                        nc.sync.dma_start(
                            out=cq[:], in_=cosq[si * P:(si + 1) * P, :])
                        sq = trig_p.tile([P, QF], f32, tag="sq")
                        nc.sync.dma_start(
                            out=sq[:], in_=sinq[si * P:(si + 1) * P, :])
                        qrot = rope_p.tile([P, QF], f16, tag="qrot")
                        qtmp = rope_p.tile([P, QF], f32, tag="qtmp")
                        rsh = rope_p.tile([P, QF], f32, tag="rsh")
                        psq_v = psq[:].rearrange("p (h t x) -> p h t x", t=2, x=32)
                        rsh_v = rsh[:].rearrange("p (h t x) -> p h t x", t=2, x=32)
                        nc.vector.tensor_copy(rsh_v[:, :, 0, :], psq_v[:, :, 1, :])
                        nc.vector.tensor_copy(rsh_v[:, :, 1, :], psq_v[:, :, 0, :])
                        nc.vector.tensor_mul(qtmp[:], psq[:], cq[:])
                        nc.vector.tensor_mul(rsh[:], rsh[:], sq[:])
                        nc.vector.tensor_add(qrot[:], qtmp[:], rsh[:])
                        ptq = ps_tr.tile([P, FT * P], f16, tag="ptr")
                        for ft in range(FT):
                            nc.tensor.transpose(
                                ptq[:, ft * P:(ft + 1) * P],
                                qrot[:, ft * P:(ft + 1) * P], ident[:])
                        nc.scalar.copy(
                            qT[:, :, si * P:(si + 1) * P],
                            ptq[:].rearrange("p (f x) -> p f x", f=FT))
                        # ---- k/v ----
                        pskv = ps_kv.tile([P, 2 * KF], f32, tag="pskv")
                        for dt in range(DT):
                            nc.tensor.matmul(
                                pskv[:], hsT_b[:, dt, j * P:(j + 1) * P],
                                wkv_sb[:, dt, :],
                                start=(dt == 0), stop=(dt == DT - 1))
                        ck = trig_p.tile([P, KF], f32, tag="ck")
                        nc.sync.dma_start(
                            out=ck[:], in_=cosk[si * P:(si + 1) * P, :])
                        sk = trig_p.tile([P, KF], f32, tag="sk")
                        nc.sync.dma_start(
                            out=sk[:], in_=sink[si * P:(si + 1) * P, :])
                        krot = rope_p.tile([P, KF], f16, tag="krot")
                        ktmp = rope_p.tile([P, KF], f32, tag="ktmp")
                        krsh = rope_p.tile([P, KF], f32, tag="krsh")
                        psk_v = pskv[:, 0:KF].rearrange(
                            "p (h t x) -> p h t x", t=2, x=32)
                        krsh_v = krsh[:].rearrange("p (h t x) -> p h t x", t=2, x=32)
                        nc.vector.tensor_copy(krsh_v[:, :, 0, :], psk_v[:, :, 1, :])
                        nc.vector.tensor_copy(krsh_v[:, :, 1, :], psk_v[:, :, 0, :])
                        nc.vector.tensor_mul(ktmp[:], pskv[:, 0:KF], ck[:])
                        nc.vector.tensor_mul(krsh[:], krsh[:], sk[:])
                        nc.vector.tensor_add(krot[:], ktmp[:], krsh[:])
                        for lkv in range(LKV):
                            pt = ps_tr.tile([P, P], f16, tag="ptr")
                            nc.tensor.transpose(
                                pt[0:HD, :],
                                krot[:, lkv * HD:(lkv + 1) * HD], ident[:])
                            nc.scalar.copy(
                                k2T[0:HD, lkv, si * P:(si + 1) * P], pt[0:HD, :])
                            nc.scalar.copy(
                                k2T[HD:P, lkv, si * P:(si + 1) * P], pt[0:HD, :])
                            nc.scalar.copy(
                                vaug[:, si, lkv * VW:lkv * VW + HD],
                                pskv[:, KF + lkv * HD:KF + (lkv + 1) * HD])

            # ---------------- phase 2+3: attention + o_proj ---------------
            with (
                tc.tile_pool(name="wo_p", bufs=1) as wo_p,
                tc.tile_pool(name="pt_p", bufs=20) as pt_p,
                tc.tile_pool(name="aT_p", bufs=2) as aT_p,
                tc.tile_pool(name="bc_p", bufs=4) as bc_p,
                tc.tile_pool(name="rd_p", bufs=4) as rd_p,
                tc.tile_pool(name="y_p", bufs=3) as y_p,
                tc.tile_pool(name="ps_s", bufs=2, space="PSUM") as ps_s,
                tc.tile_pool(name="ps_a", bufs=2, space="PSUM") as ps_a,
                tc.tile_pool(name="ps_y", bufs=2, space="PSUM") as ps_y,
            ):
                wo_sb = wo_p.tile([P, FT, D], f16)
                nc.sync.dma_start(out=wo_sb[:], in_=wo_t)

                for J in range(NJ):
                    aT = aT_p.tile([P, FT, 512], f16, tag="aT")
                    nkt = 4 * J + 4
                    for t in range(FT):          # head pair (2t, 2t+1)
                        lkv = (2 * t) // GRP
                        psa0 = ps_a.tile([VW, 512], f32, tag="psa")
                        psa1 = ps_a.tile([VW, 512], f32, tag="psa")
                        pts = []
                        for kp in range(nkt // 2):
                            pss0 = ps_s.tile([P, 1024], f32, tag="pss")
                            pss1 = ps_s.tile([P, 1024], f32, tag="pss")
                            for i in range(2):
                                kt = 2 * kp + i
                                nc.tensor.matmul(
                                    pss0[:, i * 512:(i + 1) * 512],
                                    k2T[0:HD, lkv, kt * P:(kt + 1) * P],
                                    qT[0:HD, t, J * 512:(J + 1) * 512],
                                    start=True, stop=True,
                                    tile_position=(0, 0))
                                nc.tensor.matmul(
                                    pss1[:, i * 512:(i + 1) * 512],
                                    k2T[HD:P, lkv, kt * P:(kt + 1) * P],
                                    qT[HD:P, t, J * 512:(J + 1) * 512],
                                    start=True, stop=True,
                                    tile_position=(HD, 0))
                            pt0 = pt_p.tile([P, 1024], f16, tag="pt")
                            pt1 = pt_p.tile([P, 1024], f16, tag="pt")
                            nc.scalar.activation(pt0[:], pss0[:], Exp)
                            nc.scalar.activation(pt1[:], pss1[:], Exp)
                            for i in range(2):
                                kt = 2 * kp + i
                                if kt >= 4 * J:
                                    m = kt - 4 * J
                                    nc.vector.tensor_mul(
                                        pt0[:, i * 512:(i + 1) * 512],
                                        pt0[:, i * 512:(i + 1) * 512],
                                        maskb[:, m, :])
                                    nc.vector.tensor_mul(
                                        pt1[:, i * 512:(i + 1) * 512],
                                        pt1[:, i * 512:(i + 1) * 512],
                                        maskb[:, m, :])
                            pts.append((pt0, pt1))
                        for kt in range(nkt):
                            pt0, pt1 = pts[kt // 2]
                            i = kt % 2
                            nc.tensor.matmul(
                                psa0[:], vaug[:, kt, lkv * VW:(lkv + 1) * VW],
                                pt0[:, i * 512:(i + 1) * 512],
                                start=(kt == 0), stop=(kt == nkt - 1),
                                skip_group_check=True)
                            nc.tensor.matmul(
                                psa1[:], vaug[:, kt, lkv * VW:(lkv + 1) * VW],
                                pt1[:, i * 512:(i + 1) * 512],
                                start=(kt == 0), stop=(kt == nkt - 1),
                                skip_group_check=True)
                        for psa, poff in ((psa0, 0), (psa1, HD)):
                            dn = rd_p.tile([1, 512], f32, tag="dn")
                            nc.vector.tensor_copy(dn[:], psa[HD:VW, :])
                            rc = rd_p.tile([1, 512], f32, tag="rc")
                            nc.vector.reciprocal_approx_fast(rc[:], dn[:])
                            dnb = bc_p.tile([HD, 512], f32, tag="bc")
                            nc.gpsimd.partition_broadcast(dnb[:], rc[:])
                            nc.vector.tensor_mul(
                                aT[poff:poff + HD, t, :], psa[0:HD, :], dnb[:])
                    for stl in range(4):
                        st = 4 * J + stl
                        for dn_i in range(D // 512):
                            psy = ps_y.tile([P, 512], f32, tag="psy")
                            for ft in range(FT):
                                nc.tensor.matmul(
                                    psy[:],
                                    aT[:, ft, stl * P:(stl + 1) * P],
                                    wo_sb[:, ft, dn_i * 512:(dn_i + 1) * 512],
                                    start=(ft == 0), stop=(ft == FT - 1))
                            yt = y_p.tile([P, 512], f32, tag="yt")
                            nc.vector.tensor_copy(yt[:], psy[:])
                            nc.sync.dma_start(
                                out=out_t[:, st, dn_i * 512:(dn_i + 1) * 512],
                                in_=yt[:])
    nc.compile()
    return nc


def _host_tables(cos, sin, LQ, LKV, scale):
    # sign-folded rotate-half tables, replicated per head
    hd = cos.shape[1]
    sin_pm = np.concatenate([-sin[:, :hd // 2], sin[:, hd // 2:]], axis=1)
    cosq = np.tile(cos * scale, (1, LQ)).astype(np.float32)
    sinq = np.tile(sin_pm * scale, (1, LQ)).astype(np.float32)
    cosk = np.tile(cos, (1, LKV)).astype(np.float32)
    sink = np.tile(sin_pm, (1, LKV)).astype(np.float32)
    return cosq, sinq, cosk, sink


def prepare_in_maps(hidden_states, cos, sin, Wq, Wk, Wv, Wo, LQ=8, LKV=2):
    cosq, sinq, cosk, sink = _host_tables(cos, sin, LQ, LKV, SCALE)
    nb = hidden_states.shape[0]
    hsT = [np.ascontiguousarray(hidden_states[b].T).astype(np.float16) for b in range(nb)]
    in_maps = []
    for c in range(8):
        b, g2 = c // 4, c % 4
        qs = g2 * LQ * HD
        ks = g2 * LKV * HD
        in_maps.append({
            "hsT": hsT[b],
            "wq": np.ascontiguousarray(Wq[:, qs:qs + LQ * HD]).astype(np.float16),
            "wkv": np.ascontiguousarray(
                np.concatenate([Wk[:, ks:ks + LKV * HD],
                                Wv[:, ks:ks + LKV * HD]], axis=1)).astype(np.float16),
            "wo": np.ascontiguousarray(Wo[qs:qs + LQ * HD, :]).astype(np.float16),
            "cosq": cosq, "sinq": sinq, "cosk": cosk, "sink": sink,
        })
    return in_maps


_NC_CACHE = {}


def kernel(hidden_states, attention_mask, cos, sin, Wq, Wk, Wv, Wo):
    from concourse.bass_utils import run_bass_kernel_spmd

    hidden_states = np.asarray(hidden_states, dtype=np.float32)
    cos = np.asarray(cos, dtype=np.float32)
    sin = np.asarray(sin, dtype=np.float32)
    Wq = np.asarray(Wq, dtype=np.float32)
    Wk = np.asarray(Wk, dtype=np.float32)
    Wv = np.asarray(Wv, dtype=np.float32)
    Wo = np.asarray(Wo, dtype=np.float32)

    LQ, LKV = 8, 2
    if "nc" not in _NC_CACHE:
        _NC_CACHE["nc"] = build_nc(S, D, LQ, LKV, HD)
    nc = _NC_CACHE["nc"]

    in_maps = prepare_in_maps(hidden_states, cos, sin, Wq, Wk, Wv, Wo, LQ, LKV)
    res = run_bass_kernel_spmd(nc, in_maps, core_ids=list(range(8)))
    y = np.zeros((B, S, D), dtype=np.float32)
    for c in range(8):
        y[c // 4] += res.results[c]["out"]
    return y



# revision 9
# speedup vs baseline: 1.2300x; 1.2300x over previous
"""GQA attention (RoPE, causal) + o_proj on 8 Trainium2 NeuronCores.

Sharding: 8 cores = 2 batches (DP) x 4 head-groups (TP over GQA groups).
Per core: hsT[batch] [D,S] (host-pretransposed fp16), Wq slice [D,512]
(8 q heads), Wk/Wv slice [D,128] (2 kv heads), Wo slice [512,D]. Core
computes its heads' attention and a partial o_proj output [S,D] fp16;
host sums 4 partials per batch in fp32.

Kernel (per core; fp16 matmul operands, fp32 PSUM accumulate). The
Trainium2 PE downclocks 2.4 -> 1.2 GHz whenever its pipeline gaps and
needs ~3us of continuous work to ramp back, so the whole kernel is one
fused loop over 4 sequence supertiles J (512 positions each) arranged
to keep the PE instruction stream dependency-free:

  proj(J):   q/kv projections for 4 s-tiles (natural layout), RoPE on
             DVE in fp16 (4x mode) vs fp16 trig tables, PE-transpose to
             qT/kT; v stays natural in vaug with a ones column.
  attn(J,t): per head pair: scores^T[k,q] = kT.T @ qT per 128-k-tile
             with 128-granular causal trim on the 4 diagonal k-tiles
             (trimmed chunks packed contiguously in PSUM so one exp per
             k-tile pair covers exactly the needed elements); exp on
             ACT (the only ACT work in attention sections) -> fp16 P^T;
             corner triangle masked by DVE multiply; PV interleaved one
             k-pair behind scores; A^T_aug = [V|1].T @ P^T in PSUM, row
             64 = softmax denominator; normalize via fast reciprocal +
             gpsimd partition_broadcast + DVE multiply into aT.
  oproj(J-1) four s-tile chunks interleaved: one right after proj(J)
             (covers the qT/kT copy latency) and one after each of
             attn(J,0..2), keeping the PE busy while ACT drains exp.

PSUM plan (8 banks exactly, slots time-shared via pool tag rings):
  ps_big  2x[128,1024]f32 (4 banks): q-proj psum, score tiles
  ps_sky  2x[128, 512]f32 (2 banks): kv-proj psum, q-transposes, o_proj
  ps_ax   2x[128, 512]f32 (2 banks): k-transposes, A^T accumulators
"""
import sys
import numpy as np

sys.path.insert(0, "/opt/trn_rl_repo")

B, S, D = 2, 2048, 2048
H, KVH, HD = 32, 8, 64
SCALE = HD ** -0.5
P = 128


def build_nc(S=S, D=D, LQ=8, LKV=2, HD=64):
    import concourse.bacc as bacc
    import concourse.mybir as mybir
    from concourse import tile
    from concourse.masks import make_identity

    f32 = mybir.dt.float32
    f16 = mybir.dt.float16

    QF = LQ * HD          # local q features (512)
    KF = LKV * HD         # local kv features (128)
    FT = QF // P          # q feature chunks = head pairs (4)
    DT = D // P           # contraction tiles (16)
    ST = S // P           # sequence tiles (16)
    NJ = S // 512         # q supertiles (4)
    VW = HD + 1           # v + ones column (65)
    Exp = mybir.ActivationFunctionType.Exp

    nc = bacc.Bacc(None, target_bir_lowering=False)
    hsT = nc.declare_dram_parameter("hsT", [D, S], f16, isOutput=False)
    wq = nc.declare_dram_parameter("wq", [D, QF], f16, isOutput=False)
    wkv = nc.declare_dram_parameter("wkv", [D, 2 * KF], f16, isOutput=False)
    wo = nc.declare_dram_parameter("wo", [QF, D], f16, isOutput=False)
    cosq = nc.declare_dram_parameter("cosq", [S, QF], f16, isOutput=False)
    sinq = nc.declare_dram_parameter("sinq", [S, QF], f16, isOutput=False)
    cosk = nc.declare_dram_parameter("cosk", [S, KF], f16, isOutput=False)
    sink = nc.declare_dram_parameter("sink", [S, KF], f16, isOutput=False)
    out = nc.declare_dram_parameter("out", [S, D], f16, isOutput=True)

    hsT_r = hsT.rearrange("(dt p) s -> p dt s", p=P)
    wq_t = wq.rearrange("(dt p) f -> p dt f", p=P)
    wkv_t = wkv.rearrange("(dt p) f -> p dt f", p=P)
    wo_t = wo.rearrange("(ft p) d -> p ft d", p=P)
    cosq_t = cosq.rearrange("(st p) f -> p st f", p=P)
    sinq_t = sinq.rearrange("(st p) f -> p st f", p=P)
    cosk_t = cosk.rearrange("(st p) f -> p st f", p=P)
    sink_t = sink.rearrange("(st p) f -> p st f", p=P)
    out_t = out.rearrange("(st p) d -> p st d", p=P)

    with tile.TileContext(nc) as tc:
        with tc.tile_pool(name="persist", bufs=1) as persist:
            ident = persist.tile([P, P], f16)
            maskc = persist.tile([P, P], f16)
            qT = persist.tile([P, FT, S], f16)
            kT2 = persist.tile([P, LKV, S], f16)
            vaug = persist.tile([P, ST, LKV * VW], f16)
            cq_sb = persist.tile([P, ST, QF], f16)
            sq_sb = persist.tile([P, ST, QF], f16)
            ck_sb = persist.tile([P, ST, KF], f16)
            sk_sb = persist.tile([P, ST, KF], f16)
            wq_sb = persist.tile([P, DT, QF], f16)
            wkv_sb = persist.tile([P, DT, 2 * KF], f16)
            wo_sb = persist.tile([P, FT, D], f16)

            nc.sync.dma_start(out=wq_sb[:], in_=wq_t)
            nc.sync.dma_start(out=wkv_sb[:], in_=wkv_t)

            with tc.tile_pool(name="init", bufs=1) as init_p:
                ident_f = init_p.tile([P, P], f32)
                make_identity(nc, ident_f[:])
                nc.vector.tensor_copy(ident[:], ident_f[:])
                # corner keep-mask: maskc[kr, qc] = 1 where qc >= kr
                mask_f = init_p.tile([P, P], f32)
                nc.gpsimd.memset(mask_f[:], 1.0)
                nc.gpsimd.affine_select(
                    out=mask_f[:], in_=mask_f[:],
                    compare_op=mybir.AluOpType.is_ge, fill=0.0,
                    base=0, pattern=[[1, P]], channel_multiplier=-1,
                )
                nc.vector.tensor_copy(maskc[:], mask_f[:])
                for st_i in range(ST):
                    for g in range(LKV):
                        nc.gpsimd.memset(
                            vaug[:, st_i, g * VW + HD:g * VW + HD + 1], 1.0)

            with (
                tc.tile_pool(name="hsT", bufs=2) as hsT_p,
                tc.tile_pool(name="st16", bufs=3) as st16_p,
                tc.tile_pool(name="rope", bufs=2) as rope_p,
                tc.tile_pool(name="pt_p", bufs=6) as pt_p,
                tc.tile_pool(name="aT_p", bufs=2) as aT_p,
                tc.tile_pool(name="nrm", bufs=4) as nrm_p,
                tc.tile_pool(name="bc_p", bufs=4) as bc_p,
                tc.tile_pool(name="y_p", bufs=3) as y_p,
                tc.tile_pool(name="ps_big", bufs=2, space="PSUM") as ps_big,
                tc.tile_pool(name="ps_sky", bufs=2, space="PSUM") as ps_sky,
                tc.tile_pool(name="ps_ax", bufs=2, space="PSUM") as ps_ax,
            ):
                hsT_tiles = {}
                tabs_fetched = set()

                def fetch_hsT(j):
                    if j >= NJ or j in hsT_tiles:
                        return
                    t_ = hsT_p.tile([P, DT, 512], f16, tag="hsT")
                    nc.sync.dma_start(
                        out=t_[:], in_=hsT_r[:, :, j * 512:(j + 1) * 512])
                    hsT_tiles[j] = t_

                def fetch_tabs(j):
                    if j >= NJ or j in tabs_fetched:
                        return
                    tabs_fetched.add(j)
                    sl = slice(4 * j, 4 * j + 4)
                    nc.sync.dma_start(out=cq_sb[:, sl, :], in_=cosq_t[:, sl, :])
                    nc.sync.dma_start(out=sq_sb[:, sl, :], in_=sinq_t[:, sl, :])
                    nc.sync.dma_start(out=ck_sb[:, sl, :], in_=cosk_t[:, sl, :])
                    nc.sync.dma_start(out=sk_sb[:, sl, :], in_=sink_t[:, sl, :])

                def rope16(x16, fw, rtag, ctab, stab):
                    # in/out fp16 SBUF, natural layout [s-part, features]:
                    # returns x*cos + rotate_half(x)*sin (sign pre-folded
                    # into the sin table)
                    x_v = x16[:, 0:fw].rearrange(
                        "p (h t x) -> p h t x", t=2, x=32)
                    rsh = rope_p.tile([P, fw], f16, tag=rtag + "sh")
                    r_v = rsh[:].rearrange("p (h t x) -> p h t x", t=2, x=32)
                    nc.vector.tensor_copy(r_v[:, :, 0, :], x_v[:, :, 1, :])
                    nc.vector.tensor_copy(r_v[:, :, 1, :], x_v[:, :, 0, :])
                    rot = rope_p.tile([P, fw], f16, tag=rtag)
                    nc.vector.tensor_mul(rot[:], x16[:, 0:fw], ctab)
                    nc.vector.tensor_mul(rsh[:], rsh[:], stab)
                    nc.vector.tensor_add(rot[:], rot[:], rsh[:])
                    return rot

                def proj_block(j):
                    hsT_b = hsT_tiles.pop(j)
                    fetch_hsT(j + 1)
                    fetch_tabs(j + 1)
                    pend = [None] * 4   # per-si (qrot, krot) awaiting transpose

                    def flush(si):
                        if pend[si] is None:
                            return
                        qrot, krot, st_i = pend[si]
                        pend[si] = None
                        tr = ps_ax.tile([P, QF + KF], f16, tag="ax")
                        for ft in range(FT):
                            nc.tensor.transpose(
                                tr[:, ft * P:(ft + 1) * P],
                                qrot[:, ft * P:(ft + 1) * P], ident[:])
                        nc.tensor.transpose(
                            tr[:, QF:QF + P], krot[:], ident[:])
                        scols = slice(st_i * P, (st_i + 1) * P)
                        nc.scalar.copy(
                            qT[:, :, scols],
                            tr[:, 0:QF].rearrange("p (f x) -> p f x", f=FT))
                        for g in range(LKV):
                            ksl = tr[g * HD:(g + 1) * HD, QF:QF + P]
                            nc.scalar.copy(kT2[0:HD, g, scols], ksl)
                            nc.scalar.copy(kT2[HD:P, g, scols], ksl)

                    for si in range(4):
                        st_i = 4 * j + si
                        psq = ps_big.tile([P, 1024], f32, tag="big")
                        for dt in range(DT):
                            nc.tensor.matmul(
                                psq[:, 0:QF],
                                hsT_b[:, dt, si * P:(si + 1) * P],
                                wq_sb[:, dt, :],
                                start=(dt == 0), stop=(dt == DT - 1))
                        pskv = ps_sky.tile([P, 512], f32, tag="sky")
                        for dt in range(DT):
                            nc.tensor.matmul(
                                pskv[:, 0:2 * KF],
                                hsT_b[:, dt, si * P:(si + 1) * P],
                                wkv_sb[:, dt, :],
                                start=(dt == 0), stop=(dt == DT - 1))
                        if si > 0:
                            flush(si - 1)
                        q16 = st16_p.tile([P, QF], f16, tag="q16")
                        nc.scalar.copy(q16[:], psq[:, 0:QF])
                        qrot = rope16(q16, QF, "qrot", cq_sb[:, st_i, :],
                                      sq_sb[:, st_i, :])
                        k16 = st16_p.tile([P, KF], f16, tag="k16")
                        nc.scalar.copy(k16[:], pskv[:, 0:KF])
                        krot = rope16(k16, KF, "krot", ck_sb[:, st_i, :],
                                      sk_sb[:, st_i, :])
                        for g in range(LKV):
                            nc.vector.tensor_copy(
                                vaug[:, st_i, g * VW:g * VW + HD],
                                pskv[:, KF + g * HD:KF + (g + 1) * HD])
                        pend[si] = (qrot, krot, st_i)
                    flush(3)

                def attn_block(J, t, aTt):
                    g = t // 2        # kv head for this q-head pair
                    nkt = 4 * J + 4
                    psa0 = ps_ax.tile([P, 512], f32, tag="ax")
                    psa1 = ps_ax.tile([P, 512], f32, tag="ax")
                    prev = None       # (pt0, pt1, cw) awaiting PV

                    def pv(item):
                        pt0, pt1, cw = item
                        for (kt, cc, w, qo) in cw:
                            va = vaug[:, kt, g * VW:(g + 1) * VW]
                            nc.tensor.matmul(
                                psa0[0:VW, qo:512], va, pt0[:, cc:cc + w],
                                start=(kt == 0), stop=(kt == nkt - 1),
                                skip_group_check=True)
                            nc.tensor.matmul(
                                psa1[0:VW, qo:512], va, pt1[:, cc:cc + w],
                                start=(kt == 0), stop=(kt == nkt - 1),
                                skip_group_check=True)

                    for kp in range(nkt // 2):
                        cw = []
                        c = 0
                        for i in range(2):
                            kt = 2 * kp + i
                            m = max(0, kt - 4 * J)
                            w = 512 - P * m
                            cw.append((kt, c, w, P * m))
                            c += w
                        pss0 = ps_big.tile([P, 1024], f32, tag="big")
                        pss1 = ps_big.tile([P, 1024], f32, tag="big")
                        for (kt, cc, w, qo) in cw:
                            qcol = slice(J * 512 + qo, (J + 1) * 512)
                            nc.tensor.matmul(
                                pss0[:, cc:cc + w],
                                kT2[0:HD, g, kt * P:(kt + 1) * P],
                                qT[0:HD, t, qcol],
                                start=True, stop=True)
                        for (kt, cc, w, qo) in cw:
                            qcol = slice(J * 512 + qo, (J + 1) * 512)
                            nc.tensor.matmul(
                                pss1[:, cc:cc + w],
                                kT2[HD:P, g, kt * P:(kt + 1) * P],
                                qT[HD:P, t, qcol],
                                start=True, stop=True)
                        pt0 = pt_p.tile([P, 1024], f16, tag="pt")
                        pt1 = pt_p.tile([P, 1024], f16, tag="pt")
                        nc.scalar.activation(pt0[:, 0:c], pss0[:, 0:c], Exp)
                        nc.scalar.activation(pt1[:, 0:c], pss1[:, 0:c], Exp)
                        for (kt, cc, w, qo) in cw:
                            if kt >= 4 * J:
                                nc.vector.tensor_mul(
                                    pt0[:, cc:cc + P], pt0[:, cc:cc + P],
                                    maskc[:])
                                nc.vector.tensor_mul(
                                    pt1[:, cc:cc + P], pt1[:, cc:cc + P],
                                    maskc[:])
                        if prev is not None:
                            pv(prev)
                        prev = (pt0, pt1, cw)
                    pv(prev)
                    for psa, poff in ((psa0, 0), (psa1, HD)):
                        dn = nrm_p.tile([1, 512], f32, tag="dn")
                        nc.vector.tensor_copy(dn[:], psa[HD:VW, :])
                        rc = nrm_p.tile([1, 512], f32, tag="rc")
                        nc.vector.reciprocal_approx_fast(rc[:], dn[:])
                        dnb = bc_p.tile([HD, 512], f32, tag="bc")
                        nc.gpsimd.partition_broadcast(dnb[:], rc[:])
                        nc.vector.tensor_mul(
                            aTt[poff:poff + HD, t, :], psa[0:HD, :], dnb[:])

                def oproj_chunk(aTt, st):
                    stl = st % 4
                    for dn_i in range(D // 512):
                        psy = ps_sky.tile([P, 512], f32, tag="sky")
                        for ft in range(FT):
                            nc.tensor.matmul(
                                psy[:],
                                aTt[:, ft, stl * P:(stl + 1) * P],
                                wo_sb[:, ft, dn_i * 512:(dn_i + 1) * 512],
                                start=(ft == 0), stop=(ft == FT - 1))
                        yt = y_p.tile([P, 512], f16, tag="yt")
                        nc.vector.tensor_copy(yt[:], psy[:])
                        nc.sync.dma_start(
                            out=out_t[:, st, dn_i * 512:(dn_i + 1) * 512],
                            in_=yt[:])

                fetch_tabs(0)
                fetch_hsT(0)
                wo_fetched = False
                aT_tiles = {}
                for J in range(NJ):
                    proj_block(J)
                    if not wo_fetched:
                        nc.sync.dma_start(out=wo_sb[:], in_=wo_t)
                        wo_fetched = True
                    if J >= 1:
                        oproj_chunk(aT_tiles[J - 1], 4 * (J - 1) + 0)
                    aT_tiles[J] = aT_p.tile([P, FT, 512], f16, tag="aT",
                                            name=f"aT{J}")
                    for t in range(FT):
                        attn_block(J, t, aT_tiles[J])
                        if J >= 1 and t < FT - 1:
                            oproj_chunk(aT_tiles[J - 1], 4 * (J - 1) + t + 1)
                    if J >= 2:
                        del aT_tiles[J - 2]
                for t in range(FT):
                    oproj_chunk(aT_tiles[NJ - 1], 4 * (NJ - 1) + t)
    nc.compile()
    return nc


def _host_tables(cos, sin, LQ, LKV, scale):
    # sign-folded rotate-half tables, replicated per head, fp16
    hd = cos.shape[1]
    sin_pm = np.concatenate([-sin[:, :hd // 2], sin[:, hd // 2:]], axis=1)
    cosq = np.tile(cos * scale, (1, LQ)).astype(np.float16)
    sinq = np.tile(sin_pm * scale, (1, LQ)).astype(np.float16)
    cosk = np.tile(cos, (1, LKV)).astype(np.float16)
    sink = np.tile(sin_pm, (1, LKV)).astype(np.float16)
    return cosq, sinq, cosk, sink


def prepare_in_maps(hidden_states, cos, sin, Wq, Wk, Wv, Wo, LQ=8, LKV=2):
    cos = np.asarray(cos, dtype=np.float32)
    sin = np.asarray(sin, dtype=np.float32)
    cosq, sinq, cosk, sink = _host_tables(cos, sin, LQ, LKV, SCALE)
    hidden_states = np.asarray(hidden_states, dtype=np.float32)
    Wq = np.asarray(Wq, dtype=np.float32)
    Wk = np.asarray(Wk, dtype=np.float32)
    Wv = np.asarray(Wv, dtype=np.float32)
    Wo = np.asarray(Wo, dtype=np.float32)
    nb = hidden_states.shape[0]
    hsT = [np.ascontiguousarray(hidden_states[b].T).astype(np.float16)
           for b in range(nb)]
    in_maps = []
    for c in range(8):
        b, g2 = c // 4, c % 4
        qs = g2 * LQ * HD
        ks = g2 * LKV * HD
        in_maps.append({
            "hsT": hsT[b],
            "wq": np.ascontiguousarray(Wq[:, qs:qs + LQ * HD]).astype(np.float16),
            "wkv": np.ascontiguousarray(
                np.concatenate([Wk[:, ks:ks + LKV * HD],
                                Wv[:, ks:ks + LKV * HD]], axis=1)).astype(np.float16),
            "wo": np.ascontiguousarray(Wo[qs:qs + LQ * HD, :]).astype(np.float16),
            "cosq": cosq, "sinq": sinq, "cosk": cosk, "sink": sink,
        })
    return in_maps


_NC_CACHE = {}


def kernel(hidden_states, attention_mask, cos, sin, Wq, Wk, Wv, Wo):
    from concourse.bass_utils import run_bass_kernel_spmd

    LQ, LKV = 8, 2
    if "nc" not in _NC_CACHE:
        _NC_CACHE["nc"] = build_nc(S, D, LQ, LKV, HD)
    nc = _NC_CACHE["nc"]

    in_maps = prepare_in_maps(hidden_states, cos, sin, Wq, Wk, Wv, Wo, LQ, LKV)
    res = run_bass_kernel_spmd(nc, in_maps, core_ids=list(range(8)))
    y = np.zeros((B, S, D), dtype=np.float32)
    for c in range(8):
        y[c // 4] += res.results[c]["out"].astype(np.float32)
    return y
